# revision 20
# baseline (speedup 1.0000x reference)
"""Row-sharded attention slab kernel, host-prepped fp8 Q/K/V.

Each of the 8 cores owns a [N/8, N] slab of the attention matrix.  The
host precomputes the three D x D projections in fp32 (O(N*D^2), ~5% of
total FLOPs), pre-normalizes k rows, and ships fp8 tensors: qT8 (own
rows), kT8 (all columns, pre-normalized), and v8 (row-major V).

The device does the O(N^2*D) work: score matmuls (fp8 DoubleRow),
ReLU + fp8 cast of the [N/8, N] score slab (split across ACT and DVE,
the true bottleneck), and the w@v accumulation into PSUM.  The row-sum
denominator accumulates in a separate 1-bank PSUM tile via tiny
wt @ ones matmuls, which frees enough PSUM banks to triple-buffer the
score tiles - without that, the WAR edge (score matmuls for group g+2
overwriting the tile relu(g) reads) serializes the pipeline.  Emission
is software-pipelined so those prefetch matmuls precede the w@v
matmuls of the current group in PE order.

The host then removes the diagonal term (emulating the device's fp8
rounding so the subtraction matches what the device accumulated),
divides by the row sums, and adds the residual x and the V bias (the
bias commutes out of the attention average because rows of w sum to 1).
"""

import numpy as np

import concourse.bass as bass
import concourse.bacc as bacc
import concourse.mybir as mybir
from concourse import tile
from concourse.bass_utils import run_bass_kernel_spmd

F32 = mybir.dt.float32
FP8 = mybir.dt.float8e4
AF = mybir.ActivationFunctionType
DR = mybir.MatmulPerfMode.DoubleRow

M = 8
N = 8192
D = 256

TRACE = False
LAST = None
_CACHE = {}

# ~45% of the 64 relu tiles go to DVE (1192ns/op) and the rest to ACT
# (1067ns/op); DVE also does the av evacuations, so both engines finish
# together.
N_DVE_RELU = 29


def _dve_set(total, n_dve):
    s = {int(j * total / n_dve) for j in range(n_dve)}
    # keep the final relus alternating so both engines finish together
    if total - 2 not in s:
        s.discard(max(i for i in s if i < total - 2))
        s.add(total - 2)
    return s


def build(r=N // M):
    NP_ = N // 256           # 32 column pairs (v8 major dim)
    NCH = N // 1024          # 8 streaming chunks for kT8/v8
    NG = N // 512            # 16 score groups (4 col-blocks each) per row block
    RW = 256
    NRB = r // RW            # 4 row blocks

    nc = bacc.Bacc(None)
    qT8_d = nc.declare_dram_parameter("qT8", [128, 2, r], FP8, isOutput=False)
    kT8_d = nc.declare_dram_parameter("kT8", [128, 2, N], FP8, isOutput=False)
    v8_d = nc.declare_dram_parameter("v8", [128, NP_, 2, D], FP8, isOutput=False)
    av_d = nc.declare_dram_parameter("av", [NRB, 128, 2, D], F32, isOutput=True)
    dn_d = nc.declare_dram_parameter("dn", [128, 2 * NRB], F32, isOutput=True)

    seq = [(rb, g) for rb in range(NRB) for g in range(NG)]
    dve_relu = _dve_set(len(seq), N_DVE_RELU)

    with tile.TileContext(nc, pool_alloc_mode="queue") as tc:
        with tc.tile_pool(name="pers", bufs=1) as pers, \
             tc.tile_pool(name="wtp", bufs=8) as wtp, \
             tc.tile_pool(name="avsp", bufs=2) as avsp, \
             tc.tile_pool(name="scp", bufs=3, space="PSUM") as scp, \
             tc.tile_pool(name="avp", bufs=1, space="PSUM") as avp, \
             tc.tile_pool(name="denp", bufs=1, space="PSUM") as denp:
            qT8 = pers.tile([128, 2, r], FP8, name="qT8", tag="qT8")
            kT8 = pers.tile([128, 2, N], FP8, name="kT8", tag="kT8")
            v8 = pers.tile([128, NP_, 2, D], FP8, name="v8", tag="v8")
            ones8 = pers.tile([128, 2, 1], FP8, name="ones8", tag="ones8")
            den = denp.tile([128, 2 * NRB], F32, name="den", tag="den")

            nc.vector.memset(ones8[:], 1.0)
            # warm up the ACT Relu table during the DMA fill so the ~1.3us
            # table load isn't charged to the first real score evacuation
            warm = pers.tile([128, 1, 1], FP8, name="warm", tag="warm")
            nc.scalar.activation(warm[:], ones8[:, 0:1, 0:1], AF.Relu)

            # Stream inputs on two issue queues (SP: kT8/qT8, Pool: v8) so
            # chunk delivery outpaces the ~570ns/group compute consumption;
            # small first chunks let group 0 start early.
            nc.sync.dma_start(kT8[:, :, 0:512], kT8_d[:, :, 0:512])
            nc.gpsimd.dma_start(v8[:, 0:2, :, :], v8_d[:, 0:2, :, :])
            nc.sync.dma_start(qT8[:, :, 0:RW], qT8_d[:, :, 0:RW])
            nc.gpsimd.dma_start(v8[:, 2:4, :, :], v8_d[:, 2:4, :, :])
            nc.sync.dma_start(kT8[:, :, 512:1024], kT8_d[:, :, 512:1024])
            nc.sync.dma_start(kT8[:, :, 1024:2048], kT8_d[:, :, 1024:2048])
            nc.sync.dma_start(qT8[:, :, RW:r], qT8_d[:, :, RW:r])
            for ch in range(2, NCH):
                nc.sync.dma_start(kT8[:, :, ch * 1024:(ch + 1) * 1024],
                                  kT8_d[:, :, ch * 1024:(ch + 1) * 1024])
            for ch in range(1, NCH):
                nc.gpsimd.dma_start(v8[:, ch * 4:(ch + 1) * 4, :, :],
                                    v8_d[:, ch * 4:(ch + 1) * 4, :, :])

            sc_tiles = {}

            def emit_smm(i):
                rb, g = seq[i]
                sc = scp.tile([128, 1024], F32, name="sc", tag="sc")
                sc_tiles[i] = sc
                rsl = slice(rb * RW, (rb + 1) * RW)
                for t in range(4):
                    jb = g * 4 + t
                    nc.tensor.matmul(sc[:, t * 256:(t + 1) * 256],
                                     kT8[:, :, jb * 128:(jb + 1) * 128],
                                     qT8[:, :, rsl],
                                     start=True, stop=True, perf_mode=DR)

            avs = {}
            wts = {}

            def emit_wv(i):
                rb, g = seq[i]
                wt = wts.pop(i)
                if g == 0:
                    avs[rb] = avp.tile([128, 2, D], F32, name=f"av{rb}", tag="av")
                av = avs[rb]
                for pair in range(2):
                    jp = g * 2 + pair
                    st = (g == 0 and pair == 0)
                    sp = (g == NG - 1 and pair == 1)
                    for s in range(2):
                        wsl = wt[:, pair * 2:pair * 2 + 2, s * 128:(s + 1) * 128]
                        nc.tensor.matmul(av[:, s, :], wsl, v8[:, jp, :, :],
                                         start=st, stop=sp, perf_mode=DR)
                        nc.tensor.matmul(den[:, rb * 2 + s:rb * 2 + s + 1],
                                         wsl, ones8[:],
                                         start=st, stop=sp, perf_mode=DR)
                if g == NG - 1:
                    o = avsp.tile([128, 2, D], F32, name=f"avs{rb}", tag="avs")
                    nc.vector.tensor_copy(o[:], av[:])
                    nc.sync.dma_start(av_d[rb], o[:])
                    if rb == NRB - 1:
                        od = avsp.tile([128, 2 * NRB], F32, name="dns", tag="dns")
                        nc.vector.tensor_copy(od[:], den[:])
                        nc.gpsimd.dma_start(dn_d[:], od[:])

            # Lag the w@v emission behind the relu stream so PE's in-order
            # queue isn't head-blocked by a stalled w@v; the last group of a
            # row block uses lag 1 so the evacuation (which the next row
            # block's accumulator WAR-waits on) isn't itself delayed.
            def wv_lag(i):
                return 1 if seq[i][1] == NG - 1 else 2

            wv_next = 0
            emit_smm(0)
            emit_smm(1)
            for i, (rb, g) in enumerate(seq):
                sc = sc_tiles.pop(i)
                wt = wtp.tile([128, 4, 256], FP8, name="wt", tag="wt")
                wts[i] = wt
                if i in dve_relu:
                    nc.vector.tensor_scalar_max(wt[:], sc[:], 0.0)
                else:
                    nc.scalar.activation(wt[:], sc[:], AF.Relu)
                if i + 2 < len(seq):
                    emit_smm(i + 2)
                while wv_next <= i and i - wv_next >= wv_lag(wv_next):
                    emit_wv(wv_next)
                    wv_next += 1
            while wv_next < len(seq):
                emit_wv(wv_next)
                wv_next += 1
    nc.compile()
    return nc


def _get_nc(r=N // M):
    if r not in _CACHE:
        _CACHE[r] = build(r)
    return _CACHE[r]


def _to_dr(a2d):
    c, f = a2d.shape
    return np.ascontiguousarray(a2d.reshape(2, 128, f).transpose(1, 0, 2))


def kernel(x, Wq, bq, Wk, bk, Wv, bv):
    global LAST
    np8 = mybir.dt.np(FP8)
    x = np.asarray(x, np.float32)
    n = x.shape[0]
    r = n // M

    q = x @ np.asarray(Wq, np.float32).T + np.asarray(bq, np.float32)
    k = x @ np.asarray(Wk, np.float32).T + np.asarray(bk, np.float32)
    v = x @ np.asarray(Wv, np.float32).T                      # bias added at the end
    kn = k / np.maximum(np.linalg.norm(k, axis=1, keepdims=True), 1e-12)

    q8 = q.astype(np8)
    k8 = kn.astype(np8)
    v8q = v.astype(np8)

    kT8 = _to_dr(np.ascontiguousarray(k8.T))
    v8t = np.ascontiguousarray(v8q.reshape(n // 256, 2, 128, D).transpose(2, 0, 1, 3))

    in_maps = []
    for c in range(M):
        rows = slice(c * r, (c + 1) * r)
        in_maps.append({
            "qT8": _to_dr(np.ascontiguousarray(q8[rows].T)),
            "kT8": kT8,
            "v8": v8t,
        })
    res = run_bass_kernel_spmd(_get_nc(r), in_maps, core_ids=list(range(M)), trace=TRACE)
    LAST = res

    num = np.concatenate(
        [res.results[c]["av"].transpose(0, 2, 1, 3).reshape(r, D) for c in range(M)],
        axis=0).astype(np.float32)
    # dn[p, rb*2+s] holds the row-sum for row rb*256 + s*128 + p
    den = np.concatenate(
        [res.results[c]["dn"].reshape(128, r // 256, 2).transpose(1, 2, 0).reshape(r)
         for c in range(M)], axis=0).astype(np.float32)

    # Remove the diagonal term, emulating the device's fp8 rounding of the
    # relu'd score and of v so the subtraction cancels what was accumulated.
    sdiag = np.maximum((q8.astype(np.float32) * k8.astype(np.float32)).sum(axis=1), 0.0)
    wdiag = sdiag.astype(np8).astype(np.float32)
    num = num - wdiag[:, None] * v8q.astype(np.float32)
    den = den - wdiag

    out = num / np.maximum(den, 1e-12)[:, None] + x + np.asarray(bv, np.float32)
    return np.ascontiguousarray(out, dtype=np.float32)


# revision 38
# speedup vs baseline: 1.0117x; 1.0117x over previous
"""Row-sharded attention slab kernel, host-prepped fp8 Q/K/V.

Each of the 8 cores owns a [N/8, N] slab of the attention matrix.  The
host precomputes the three D x D projections in fp32 (O(N*D^2), ~5% of
total FLOPs), pre-normalizes k rows, and ships fp8 tensors: qT8 (own
rows), kT8 (all columns, pre-normalized), and v8 (row-major V).

The device does the O(N^2*D) work: score matmuls (fp8 DoubleRow),
ReLU + fp8 cast of the [N/8, N] score slab (split across ACT and DVE,
the true bottleneck), and the w@v accumulation into PSUM.  The row-sum
denominator accumulates in a separate 1-bank PSUM tile via tiny
wt @ ones matmuls, which frees enough PSUM banks to triple-buffer the
score tiles - without that, the WAR edge (score matmuls for group g+2
overwriting the tile relu(g) reads) serializes the pipeline.  Emission
is software-pipelined so those prefetch matmuls precede the w@v
matmuls of the current group in PE order.

The host then removes the diagonal term (emulating the device's fp8
rounding so the subtraction matches what the device accumulated),
divides by the row sums, and adds the residual x and the V bias (the
bias commutes out of the attention average because rows of w sum to 1).
"""

import numpy as np

import concourse.bass as bass
import concourse.bacc as bacc
import concourse.mybir as mybir
from concourse import tile
from concourse.bass_utils import run_bass_kernel_spmd

F32 = mybir.dt.float32
FP8 = mybir.dt.float8e4
AF = mybir.ActivationFunctionType
DR = mybir.MatmulPerfMode.DoubleRow

M = 8
N = 8192
D = 256

TRACE = False
LAST = None
_CACHE = {}

# ~45% of the relu tiles go to DVE (1192ns/op) and the rest to ACT
# (1067ns/op) so both engines finish together.
N_DVE_RELU = 29
# number of half-width (2 col-block) groups at the start of rb0 / end of
# the last rb: shortens pipeline fill and drain at the cost of one extra
# per-op bubble each
N_SMALL_HEAD = 0
N_SMALL_TAIL = 0
WVLAG_BASE = 2
WT_BUFS = 8


def _dve_set(total, n_dve):
    s = {int(j * total / n_dve) for j in range(n_dve)}
    # keep the final relus alternating so both engines finish together
    if total - 2 not in s:
        s.discard(max(i for i in s if i < total - 2))
        s.add(total - 2)
    return s


def build(r=N // M):
    NP_ = N // 256           # 32 column pairs (v8 major dim)
    NCH = N // 1024          # 8 streaming chunks for kT8/v8
    NG = N // 512            # 16 score groups (4 col-blocks each) per row block
    RW = 256
    NRB = r // RW            # 4 row blocks

    nc = bacc.Bacc(None)
    qT8_d = nc.declare_dram_parameter("qT8", [128, 2, r], FP8, isOutput=False)
    kT8_d = nc.declare_dram_parameter("kT8", [128, 2, N], FP8, isOutput=False)
    v8_d = nc.declare_dram_parameter("v8", [128, NP_, 2, D], FP8, isOutput=False)
    # av_d[rb, p, s, :] = [num | den] for row rb*256 + s*128 + p
    av_d = nc.declare_dram_parameter("av", [NRB, 128, 2, D + 1], F32, isOutput=True)

    # groups: (rb, first col-block, n col-blocks); narrow groups at the two
    # ends of the run shorten fill/drain
    seq = []
    for rb in range(NRB):
        jbs = []
        j = 0
        if rb == 0:
            for _ in range(N_SMALL_HEAD):
                jbs.append((j, 2))
                j += 2
        tail = [];
        jt = 4 * NG
        if rb == NRB - 1:
            for _ in range(N_SMALL_TAIL):
                jt -= 2
                tail.append((jt, 2))
            tail.reverse()
        while j < jt:
            jbs.append((j, 4))
            j += 4
        for j0, njb in jbs + tail:
            seq.append((rb, j0, njb))
    dve_relu = _dve_set(len(seq), N_DVE_RELU)

    with tile.TileContext(nc, pool_alloc_mode="queue") as tc:
        with tc.tile_pool(name="pers", bufs=1) as pers, \
             tc.tile_pool(name="wtp", bufs=WT_BUFS) as wtp, \
             tc.tile_pool(name="avsp", bufs=2) as avsp, \
             tc.tile_pool(name="scp", bufs=3, space="PSUM") as scp, \
             tc.tile_pool(name="avp", bufs=1, space="PSUM") as avp, \
             tc.tile_pool(name="denp", bufs=1, space="PSUM") as denp:
            qT8 = pers.tile([128, 2, r], FP8, name="qT8", tag="qT8")
            kT8 = pers.tile([128, 2, N], FP8, name="kT8", tag="kT8")
            v8 = pers.tile([128, NP_, 2, D], FP8, name="v8", tag="v8")
            ones8 = pers.tile([128, 2, 1], FP8, name="ones8", tag="ones8")
            den = denp.tile([128, 2 * NRB], F32, name="den", tag="den")

            nc.vector.memset(ones8[:], 1.0)
            # warm up the ACT Relu table during the DMA fill so the ~1.3us
            # table load isn't charged to the first real score evacuation
            warm = pers.tile([128, 1, 1], FP8, name="warm", tag="warm")
            nc.scalar.activation(warm[:], ones8[:, 0:1, 0:1], AF.Relu)

            # Stream inputs on two issue queues (SP: kT8/qT8, Pool: v8) so
            # chunk delivery outpaces the ~570ns/group compute consumption;
            # small first chunks let group 0 start early.
            nc.sync.dma_start(kT8[:, :, 0:512], kT8_d[:, :, 0:512])
            nc.gpsimd.dma_start(v8[:, 0:2, :, :], v8_d[:, 0:2, :, :])
            nc.sync.dma_start(qT8[:, :, 0:RW], qT8_d[:, :, 0:RW])
            nc.gpsimd.dma_start(v8[:, 2:4, :, :], v8_d[:, 2:4, :, :])
            nc.sync.dma_start(kT8[:, :, 512:1024], kT8_d[:, :, 512:1024])
            nc.sync.dma_start(kT8[:, :, 1024:2048], kT8_d[:, :, 1024:2048])
            nc.sync.dma_start(qT8[:, :, RW:r], qT8_d[:, :, RW:r])
            for ch in range(2, NCH):
                nc.sync.dma_start(kT8[:, :, ch * 1024:(ch + 1) * 1024],
                                  kT8_d[:, :, ch * 1024:(ch + 1) * 1024])
            for ch in range(1, NCH):
                nc.gpsimd.dma_start(v8[:, ch * 4:(ch + 1) * 4, :, :],
                                    v8_d[:, ch * 4:(ch + 1) * 4, :, :])

            sc_tiles = {}

            def emit_smm(i):
                rb, j0, njb = seq[i]
                sc = scp.tile([128, njb * 256], F32, name="sc", tag="sc")
                sc_tiles[i] = sc
                rsl = slice(rb * RW, (rb + 1) * RW)
                for t in range(njb):
                    jb = j0 + t
                    nc.tensor.matmul(sc[:, t * 256:(t + 1) * 256],
                                     kT8[:, :, jb * 128:(jb + 1) * 128],
                                     qT8[:, :, rsl],
                                     start=True, stop=True, perf_mode=DR)

            avs = {}
            wts = {}

            def emit_wv(i):
                rb, j0, njb = seq[i]
                wt = wts.pop(i)
                if j0 == 0:
                    avs[rb] = avp.tile([128, 2, D], F32, name=f"av{rb}", tag="av")
                av = avs[rb]
                for pair in range(njb // 2):
                    jp = j0 // 2 + pair
                    st = (j0 == 0 and pair == 0)
                    sp = (j0 + njb == 4 * NG and pair == njb // 2 - 1)
                    for s in range(2):
                        wsl = wt[:, pair * 2:pair * 2 + 2, s * 128:(s + 1) * 128]
                        nc.tensor.matmul(av[:, s, :], wsl, v8[:, jp, :, :],
                                         start=st, stop=sp, perf_mode=DR)
                        nc.tensor.matmul(den[:, rb * 2 + s:rb * 2 + s + 1],
                                         wsl, ones8[:],
                                         start=st, stop=sp, perf_mode=DR)
                if j0 + njb == 4 * NG:
                    # split the evacuation across both engines (separate
                    # tiles - a shared tile's write-write dep would
                    # serialize them) to halve the latency before the next
                    # row block may reuse the av bank
                    o0 = avsp.tile([128, 1, D + 1], F32, name=f"avsa{rb}", tag="avsa")
                    o1 = avsp.tile([128, 1, D + 1], F32, name=f"avsb{rb}", tag="avsb")
                    nc.vector.tensor_copy(o0[:, 0, 0:D], av[:, 0, :])
                    nc.vector.tensor_copy(o0[:, 0, D:D + 1], den[:, rb * 2:rb * 2 + 1])
                    nc.scalar.activation(o1[:, 0, 0:D], av[:, 1, :], AF.Copy)
                    nc.scalar.activation(o1[:, 0, D:D + 1],
                                         den[:, rb * 2 + 1:rb * 2 + 2], AF.Copy)
                    nc.sync.dma_start(av_d[rb, :, 0:1, :], o0[:])
                    nc.sync.dma_start(av_d[rb, :, 1:2, :], o1[:])

            # Lag the w@v emission behind the relu stream so PE's in-order
            # queue isn't head-blocked by a stalled w@v; the last group of a
            # row block uses lag 1 so the evacuation (which the next row
            # block's accumulator WAR-waits on) isn't itself delayed.
            def wv_lag(i):
                return 1 if seq[i][1] + seq[i][2] == 4 * NG else WVLAG_BASE

            wv_next = 0
            emit_smm(0)
            emit_smm(1)
            for i, (rb, j0, njb) in enumerate(seq):
                sc = sc_tiles.pop(i)
                wt = wtp.tile([128, njb, 256], FP8, name="wt", tag="wt")
                wts[i] = wt
                if i in dve_relu:
                    nc.vector.tensor_scalar_max(wt[:], sc[:], 0.0)
                else:
                    nc.scalar.activation(wt[:], sc[:], AF.Relu)
                if i + 2 < len(seq):
                    emit_smm(i + 2)
                while wv_next <= i and i - wv_next >= wv_lag(wv_next):
                    emit_wv(wv_next)
                    wv_next += 1
            while wv_next < len(seq):
                emit_wv(wv_next)
                wv_next += 1
    nc.compile()
    return nc


def _get_nc(r=N // M):
    if r not in _CACHE:
        _CACHE[r] = build(r)
    return _CACHE[r]


def _to_dr(a2d):
    c, f = a2d.shape
    return np.ascontiguousarray(a2d.reshape(2, 128, f).transpose(1, 0, 2))


def kernel(x, Wq, bq, Wk, bk, Wv, bv):
    global LAST
    np8 = mybir.dt.np(FP8)
    x = np.asarray(x, np.float32)
    n = x.shape[0]
    r = n // M

    q = x @ np.asarray(Wq, np.float32).T + np.asarray(bq, np.float32)
    k = x @ np.asarray(Wk, np.float32).T + np.asarray(bk, np.float32)
    v = x @ np.asarray(Wv, np.float32).T                      # bias added at the end
    kn = k / np.maximum(np.linalg.norm(k, axis=1, keepdims=True), 1e-12)

    q8 = q.astype(np8)
    k8 = kn.astype(np8)
    v8q = v.astype(np8)

    kT8 = _to_dr(np.ascontiguousarray(k8.T))
    v8t = np.ascontiguousarray(v8q.reshape(n // 256, 2, 128, D).transpose(2, 0, 1, 3))

    in_maps = []
    for c in range(M):
        rows = slice(c * r, (c + 1) * r)
        in_maps.append({
            "qT8": _to_dr(np.ascontiguousarray(q8[rows].T)),
            "kT8": kT8,
            "v8": v8t,
        })
    res = run_bass_kernel_spmd(_get_nc(r), in_maps, core_ids=list(range(M)), trace=TRACE)
    LAST = res

    av = np.concatenate(
        [res.results[c]["av"].transpose(0, 2, 1, 3).reshape(r, D + 1) for c in range(M)],
        axis=0).astype(np.float32)
    num = av[:, 0:D]
    den = av[:, D]

    # Remove the diagonal term, emulating the device's fp8 rounding of the
    # relu'd score and of v so the subtraction cancels what was accumulated.
    sdiag = np.maximum((q8.astype(np.float32) * k8.astype(np.float32)).sum(axis=1), 0.0)
    wdiag = sdiag.astype(np8).astype(np.float32)
    num = num - wdiag[:, None] * v8q.astype(np.float32)
    den = den - wdiag

    out = num / np.maximum(den, 1e-12)[:, None] + x + np.asarray(bv, np.float32)
    return np.ascontiguousarray(out, dtype=np.float32)


# revision 48
# speedup vs baseline: 1.0146x; 1.0029x over previous
"""Row-sharded attention slab kernel, host-prepped fp8 Q/K/V.

Each of the 8 cores owns a [N/8, N] slab of the attention matrix.  The
host precomputes the three D x D projections in fp32 (O(N*D^2), ~5% of
total FLOPs), pre-normalizes k rows, and ships fp8 tensors: qT8 (own
rows), kT8 (all columns, pre-normalized), and v8 (row-major V).

The device does the O(N^2*D) work: score matmuls (fp8 DoubleRow),
ReLU + fp8 cast of the [N/8, N] score slab (split across ACT and DVE,
the true bottleneck), and the w@v accumulation into PSUM.  The row-sum
denominator accumulates in a separate 1-bank PSUM tile via tiny
wt @ ones matmuls, which frees enough PSUM banks to triple-buffer the
score tiles - without that, the WAR edge (score matmuls for group g+2
overwriting the tile relu(g) reads) serializes the pipeline.  Emission
is software-pipelined so those prefetch matmuls precede the w@v
matmuls of the current group in PE order.

The host then removes the diagonal term (emulating the device's fp8
rounding so the subtraction matches what the device accumulated),
divides by the row sums, and adds the residual x and the V bias (the
bias commutes out of the attention average because rows of w sum to 1).
"""

import numpy as np

import concourse.bass as bass
import concourse.bacc as bacc
import concourse.mybir as mybir
from concourse import tile
from concourse.bass_utils import run_bass_kernel_spmd

F32 = mybir.dt.float32
FP8 = mybir.dt.float8e4
AF = mybir.ActivationFunctionType
DR = mybir.MatmulPerfMode.DoubleRow

M = 8
N = 8192
D = 256

TRACE = False
LAST = None
_CACHE = {}

# ~45% of the relu tiles go to DVE (1192ns/op) and the rest to ACT
# (1067ns/op) so both engines finish together.
N_DVE_RELU = 29
# number of half-width (2 col-block) groups at the start of rb0 / end of
# the last rb: shortens pipeline fill and drain at the cost of one extra
# per-op bubble each
N_SMALL_HEAD = 0
N_SMALL_TAIL = 0
WVLAG_BASE = 2
WT_BUFS = 8


def _dve_set(total, n_dve):
    s = {int(j * total / n_dve) for j in range(n_dve)}
    # keep the final relus alternating so both engines finish together
    if total - 2 not in s:
        s.discard(max(i for i in s if i < total - 2))
        s.add(total - 2)
    return s


def build(r=N // M):
    NP_ = N // 256           # 32 column pairs (v8 major dim)
    NCH = N // 1024          # 8 streaming chunks for kT8/v8
    NG = N // 512            # 16 score groups (4 col-blocks each) per row block
    RW = 256
    NRB = r // RW            # 4 row blocks

    nc = bacc.Bacc(None)
    qT8_d = nc.declare_dram_parameter("qT8", [128, 2, r], FP8, isOutput=False)
    kT8_d = nc.declare_dram_parameter("kT8", [128, 2, N], FP8, isOutput=False)
    v8_d = nc.declare_dram_parameter("v8", [128, NP_, 2, D], FP8, isOutput=False)
    # av_d[rb, p, s, :] = [num | den] for row rb*256 + s*128 + p
    av_d = nc.declare_dram_parameter("av", [NRB, 128, 2, D + 1], F32, isOutput=True)

    # groups: (rb, first col-block, n col-blocks); narrow groups at the two
    # ends of the run shorten fill/drain
    seq = []
    for rb in range(NRB):
        jbs = []
        j = 0
        if rb == 0:
            for _ in range(N_SMALL_HEAD):
                jbs.append((j, 2))
                j += 2
        tail = [];
        jt = 4 * NG
        if rb == NRB - 1:
            for _ in range(N_SMALL_TAIL):
                jt -= 2
                tail.append((jt, 2))
            tail.reverse()
        while j < jt:
            jbs.append((j, 4))
            j += 4
        for j0, njb in jbs + tail:
            seq.append((rb, j0, njb))
    dve_relu = _dve_set(len(seq), N_DVE_RELU)

    with tile.TileContext(nc, pool_alloc_mode="queue") as tc:
        with tc.tile_pool(name="pers", bufs=1) as pers, \
             tc.tile_pool(name="wtp", bufs=WT_BUFS) as wtp, \
             tc.tile_pool(name="avsp", bufs=2) as avsp, \
             tc.tile_pool(name="scp", bufs=3, space="PSUM") as scp, \
             tc.tile_pool(name="avp", bufs=1, space="PSUM") as avp, \
             tc.tile_pool(name="denp", bufs=1, space="PSUM") as denp:
            qT8 = pers.tile([128, 2, r], FP8, name="qT8", tag="qT8")
            kT8 = pers.tile([128, 2, N], FP8, name="kT8", tag="kT8")
            v8 = pers.tile([128, NP_, 2, D], FP8, name="v8", tag="v8")
            ones8 = pers.tile([128, 2, 1], FP8, name="ones8", tag="ones8")
            den = denp.tile([128, 2 * NRB], F32, name="den", tag="den")

            nc.vector.memset(ones8[:], 1.0)
            # warm up the ACT Relu table during the DMA fill so the ~1.3us
            # table load isn't charged to the first real score evacuation
            warm = pers.tile([128, 1, 1], FP8, name="warm", tag="warm")
            nc.scalar.activation(warm[:], ones8[:, 0:1, 0:1], AF.Relu)

            # Stream inputs on two issue queues (SP: kT8/qT8, Pool: v8) so
            # chunk delivery outpaces the ~570ns/group compute consumption;
            # small first chunks let group 0 start early.
            nc.sync.dma_start(kT8[:, :, 0:512], kT8_d[:, :, 0:512])
            nc.gpsimd.dma_start(v8[:, 0:2, :, :], v8_d[:, 0:2, :, :])
            nc.sync.dma_start(qT8[:, :, 0:RW], qT8_d[:, :, 0:RW])
            nc.gpsimd.dma_start(v8[:, 2:4, :, :], v8_d[:, 2:4, :, :])
            nc.sync.dma_start(kT8[:, :, 512:1024], kT8_d[:, :, 512:1024])
            nc.sync.dma_start(kT8[:, :, 1024:2048], kT8_d[:, :, 1024:2048])
            nc.sync.dma_start(kT8[:, :, 2048:3072], kT8_d[:, :, 2048:3072])
            nc.sync.dma_start(qT8[:, :, RW:r], qT8_d[:, :, RW:r])
            for ch in range(3, NCH):
                nc.sync.dma_start(kT8[:, :, ch * 1024:(ch + 1) * 1024],
                                  kT8_d[:, :, ch * 1024:(ch + 1) * 1024])
            for ch in range(1, NCH):
                nc.gpsimd.dma_start(v8[:, ch * 4:(ch + 1) * 4, :, :],
                                    v8_d[:, ch * 4:(ch + 1) * 4, :, :])

            sc_tiles = {}

            def emit_smm(i):
                rb, j0, njb = seq[i]
                sc = scp.tile([128, njb * 256], F32, name="sc", tag="sc")
                sc_tiles[i] = sc
                rsl = slice(rb * RW, (rb + 1) * RW)
                for t in range(njb):
                    jb = j0 + t
                    nc.tensor.matmul(sc[:, t * 256:(t + 1) * 256],
                                     kT8[:, :, jb * 128:(jb + 1) * 128],
                                     qT8[:, :, rsl],
                                     start=True, stop=True, perf_mode=DR)

            avs = {}
            wts = {}

            def emit_wv(i):
                rb, j0, njb = seq[i]
                wt = wts.pop(i)
                if j0 == 0:
                    avs[rb] = avp.tile([128, 2, D], F32, name=f"av{rb}", tag="av")
                av = avs[rb]
                for pair in range(njb // 2):
                    jp = j0 // 2 + pair
                    st = (j0 == 0 and pair == 0)
                    sp = (j0 + njb == 4 * NG and pair == njb // 2 - 1)
                    for s in range(2):
                        wsl = wt[:, pair * 2:pair * 2 + 2, s * 128:(s + 1) * 128]
                        nc.tensor.matmul(av[:, s, :], wsl, v8[:, jp, :, :],
                                         start=st, stop=sp, perf_mode=DR)
                        nc.tensor.matmul(den[:, rb * 2 + s:rb * 2 + s + 1],
                                         wsl, ones8[:],
                                         start=st, stop=sp, perf_mode=DR)
                if j0 + njb == 4 * NG:
                    # split the evacuation across both engines (separate
                    # tiles - a shared tile's write-write dep would
                    # serialize them) to halve the latency before the next
                    # row block may reuse the av bank
                    o0 = avsp.tile([128, 1, D + 1], F32, name=f"avsa{rb}", tag="avsa")
                    o1 = avsp.tile([128, 1, D + 1], F32, name=f"avsb{rb}", tag="avsb")
                    nc.vector.tensor_copy(o0[:, 0, 0:D], av[:, 0, :])
                    nc.vector.tensor_copy(o0[:, 0, D:D + 1], den[:, rb * 2:rb * 2 + 1])
                    nc.scalar.activation(o1[:, 0, 0:D], av[:, 1, :], AF.Copy)
                    nc.scalar.activation(o1[:, 0, D:D + 1],
                                         den[:, rb * 2 + 1:rb * 2 + 2], AF.Copy)
                    nc.sync.dma_start(av_d[rb, :, 0:1, :], o0[:])
                    nc.gpsimd.dma_start(av_d[rb, :, 1:2, :], o1[:])

            # Lag the w@v emission behind the relu stream so PE's in-order
            # queue isn't head-blocked by a stalled w@v; the last group of a
            # row block uses lag 1 so the evacuation (which the next row
            # block's accumulator WAR-waits on) isn't itself delayed.
            def wv_lag(i):
                return 1 if seq[i][1] + seq[i][2] == 4 * NG else WVLAG_BASE

            wv_next = 0
            emit_smm(0)
            emit_smm(1)
            for i, (rb, j0, njb) in enumerate(seq):
                sc = sc_tiles.pop(i)
                wt = wtp.tile([128, njb, 256], FP8, name="wt", tag="wt")
                wts[i] = wt
                if i in dve_relu:
                    nc.vector.tensor_scalar_max(wt[:], sc[:], 0.0)
                else:
                    nc.scalar.activation(wt[:], sc[:], AF.Relu)
                if i + 2 < len(seq):
                    emit_smm(i + 2)
                while wv_next <= i and i - wv_next >= wv_lag(wv_next):
                    emit_wv(wv_next)
                    wv_next += 1
            while wv_next < len(seq):
                emit_wv(wv_next)
                wv_next += 1
    nc.compile()
    return nc


def _get_nc(r=N // M):
    if r not in _CACHE:
        _CACHE[r] = build(r)
    return _CACHE[r]


def _to_dr(a2d):
    c, f = a2d.shape
    return np.ascontiguousarray(a2d.reshape(2, 128, f).transpose(1, 0, 2))


def kernel(x, Wq, bq, Wk, bk, Wv, bv):
    global LAST
    np8 = mybir.dt.np(FP8)
    x = np.asarray(x, np.float32)
    n = x.shape[0]
    r = n // M

    q = x @ np.asarray(Wq, np.float32).T + np.asarray(bq, np.float32)
    k = x @ np.asarray(Wk, np.float32).T + np.asarray(bk, np.float32)
    v = x @ np.asarray(Wv, np.float32).T                      # bias added at the end
    kn = k / np.maximum(np.linalg.norm(k, axis=1, keepdims=True), 1e-12)

    q8 = q.astype(np8)
    k8 = kn.astype(np8)
    v8q = v.astype(np8)

    kT8 = _to_dr(np.ascontiguousarray(k8.T))
    v8t = np.ascontiguousarray(v8q.reshape(n // 256, 2, 128, D).transpose(2, 0, 1, 3))

    in_maps = []
    for c in range(M):
        rows = slice(c * r, (c + 1) * r)
        in_maps.append({
            "qT8": _to_dr(np.ascontiguousarray(q8[rows].T)),
            "kT8": kT8,
            "v8": v8t,
        })
    res = run_bass_kernel_spmd(_get_nc(r), in_maps, core_ids=list(range(M)), trace=TRACE)
    LAST = res

    av = np.concatenate(
        [res.results[c]["av"].transpose(0, 2, 1, 3).reshape(r, D + 1) for c in range(M)],
        axis=0).astype(np.float32)
    num = av[:, 0:D]
    den = av[:, D]

    # Remove the diagonal term, emulating the device's fp8 rounding of the
    # relu'd score and of v so the subtraction cancels what was accumulated.
    sdiag = np.maximum((q8.astype(np.float32) * k8.astype(np.float32)).sum(axis=1), 0.0)
    wdiag = sdiag.astype(np8).astype(np.float32)
    num = num - wdiag[:, None] * v8q.astype(np.float32)
    den = den - wdiag

    out = num / np.maximum(den, 1e-12)[:, None] + x + np.asarray(bv, np.float32)
    return np.ascontiguousarray(out, dtype=np.float32)


# revision 51
# speedup vs baseline: 1.0311x; 1.0162x over previous
"""Row-sharded attention slab kernel, host-prepped fp8 Q/K/V.

Each of the 8 cores owns a [N/8, N] slab of the attention matrix.  The
host precomputes the three D x D projections in fp32 (O(N*D^2), ~5% of
total FLOPs), pre-normalizes k rows, and ships fp8 tensors: qT8 (own
rows), kT8 (all columns, pre-normalized), and v8 (row-major V).

The device does the O(N^2*D) work: score matmuls (fp8 DoubleRow),
ReLU + fp8 cast of the [N/8, N] score slab (split across ACT and DVE,
the true bottleneck), and the w@v accumulation into PSUM.  The row-sum
denominator accumulates in a separate 1-bank PSUM tile via tiny
wt @ ones matmuls, which frees enough PSUM banks to triple-buffer the
score tiles - without that, the WAR edge (score matmuls for group g+2
overwriting the tile relu(g) reads) serializes the pipeline.  Emission
is software-pipelined so those prefetch matmuls precede the w@v
matmuls of the current group in PE order.

The host then removes the diagonal term (emulating the device's fp8
rounding so the subtraction matches what the device accumulated),
divides by the row sums, and adds the residual x and the V bias (the
bias commutes out of the attention average because rows of w sum to 1).
"""

import numpy as np

import concourse.bass as bass
import concourse.bacc as bacc
import concourse.mybir as mybir
from concourse import tile
from concourse.bass_utils import run_bass_kernel_spmd

F32 = mybir.dt.float32
FP8 = mybir.dt.float8e4
AF = mybir.ActivationFunctionType
DR = mybir.MatmulPerfMode.DoubleRow

M = 8
N = 8192
D = 256

TRACE = False
LAST = None
_CACHE = {}

# ~45% of the relu tiles go to DVE (1192ns/op) and the rest to ACT
# (1067ns/op) so both engines finish together.
N_DVE_RELU = 29
# number of half-width (2 col-block) groups at the start of rb0 / end of
# the last rb: shortens pipeline fill and drain at the cost of one extra
# per-op bubble each
N_SMALL_HEAD = 0
N_SMALL_TAIL = 0
WVLAG_BASE = 2
WT_BUFS = 8


DVE_PHASE = 0.46
# engine assignment found by randomized local search over the cost model
DVE_SET = {1, 3, 5, 7, 9, 12, 14, 16, 18, 20, 23, 25, 27, 29, 31, 33,
           36, 38, 40, 42, 44, 47, 49, 51, 53, 56, 58, 60, 62}


def _dve_set(total, n_dve):
    s = {int((j + DVE_PHASE) * total / n_dve) % total for j in range(n_dve)}
    while len(s) < n_dve:
        s.add(max(set(range(total)) - s))
    # keep the final relus alternating so both engines finish together
    if total - 2 not in s:
        s.discard(max(i for i in s if i < total - 2))
        s.add(total - 2)
    return s


def build(r=N // M):
    NP_ = N // 256           # 32 column pairs (v8 major dim)
    NCH = N // 1024          # 8 streaming chunks for kT8/v8
    NG = N // 512            # 16 score groups (4 col-blocks each) per row block
    RW = 256
    NRB = r // RW            # 4 row blocks

    nc = bacc.Bacc(None)
    qT8_d = nc.declare_dram_parameter("qT8", [128, 2, r], FP8, isOutput=False)
    kT8_d = nc.declare_dram_parameter("kT8", [128, 2, N], FP8, isOutput=False)
    v8_d = nc.declare_dram_parameter("v8", [128, NP_, 2, D], FP8, isOutput=False)
    # av_d[rb, p, s, :] = [num | den] for row rb*256 + s*128 + p
    av_d = nc.declare_dram_parameter("av", [NRB, 128, 2, D + 1], F32, isOutput=True)

    # groups: (rb, first col-block, n col-blocks); narrow groups at the two
    # ends of the run shorten fill/drain
    seq = []
    for rb in range(NRB):
        jbs = []
        j = 0
        if rb == 0:
            for _ in range(N_SMALL_HEAD):
                jbs.append((j, 2))
                j += 2
        tail = [];
        jt = 4 * NG
        if rb == NRB - 1:
            for _ in range(N_SMALL_TAIL):
                jt -= 2
                tail.append((jt, 2))
            tail.reverse()
        while j < jt:
            jbs.append((j, 4))
            j += 4
        for j0, njb in jbs + tail:
            seq.append((rb, j0, njb))
    dve_relu = DVE_SET if DVE_SET is not None else _dve_set(len(seq), N_DVE_RELU)

    with tile.TileContext(nc, pool_alloc_mode="queue") as tc:
        with tc.tile_pool(name="pers", bufs=1) as pers, \
             tc.tile_pool(name="wtp", bufs=WT_BUFS) as wtp, \
             tc.tile_pool(name="avsp", bufs=2) as avsp, \
             tc.tile_pool(name="scp", bufs=3, space="PSUM") as scp, \
             tc.tile_pool(name="avp", bufs=1, space="PSUM") as avp, \
             tc.tile_pool(name="denp", bufs=1, space="PSUM") as denp:
            qT8 = pers.tile([128, 2, r], FP8, name="qT8", tag="qT8")
            kT8 = pers.tile([128, 2, N], FP8, name="kT8", tag="kT8")
            v8 = pers.tile([128, NP_, 2, D], FP8, name="v8", tag="v8")
            ones8 = pers.tile([128, 2, 1], FP8, name="ones8", tag="ones8")
            den = denp.tile([128, 2 * NRB], F32, name="den", tag="den")

            nc.vector.memset(ones8[:], 1.0)
            # warm up the ACT Relu table during the DMA fill so the ~1.3us
            # table load isn't charged to the first real score evacuation
            warm = pers.tile([128, 1, 1], FP8, name="warm", tag="warm")
            nc.scalar.activation(warm[:], ones8[:, 0:1, 0:1], AF.Relu)

            # Stream inputs on two issue queues (SP: kT8/qT8, Pool: v8) so
            # chunk delivery outpaces the ~570ns/group compute consumption;
            # small first chunks let group 0 start early.
            nc.sync.dma_start(kT8[:, :, 0:512], kT8_d[:, :, 0:512])
            nc.gpsimd.dma_start(v8[:, 0:2, :, :], v8_d[:, 0:2, :, :])
            nc.sync.dma_start(qT8[:, :, 0:RW], qT8_d[:, :, 0:RW])
            nc.gpsimd.dma_start(v8[:, 2:4, :, :], v8_d[:, 2:4, :, :])
            nc.sync.dma_start(kT8[:, :, 512:1024], kT8_d[:, :, 512:1024])
            nc.sync.dma_start(kT8[:, :, 1024:2048], kT8_d[:, :, 1024:2048])
            nc.sync.dma_start(kT8[:, :, 2048:3072], kT8_d[:, :, 2048:3072])
            nc.sync.dma_start(qT8[:, :, RW:r], qT8_d[:, :, RW:r])
            for ch in range(3, NCH):
                nc.sync.dma_start(kT8[:, :, ch * 1024:(ch + 1) * 1024],
                                  kT8_d[:, :, ch * 1024:(ch + 1) * 1024])
            for ch in range(1, NCH):
                nc.gpsimd.dma_start(v8[:, ch * 4:(ch + 1) * 4, :, :],
                                    v8_d[:, ch * 4:(ch + 1) * 4, :, :])

            sc_tiles = {}

            def emit_smm(i):
                rb, j0, njb = seq[i]
                sc = scp.tile([128, njb * 256], F32, name="sc", tag="sc")
                sc_tiles[i] = sc
                rsl = slice(rb * RW, (rb + 1) * RW)
                for t in range(njb):
                    jb = j0 + t
                    nc.tensor.matmul(sc[:, t * 256:(t + 1) * 256],
                                     kT8[:, :, jb * 128:(jb + 1) * 128],
                                     qT8[:, :, rsl],
                                     start=True, stop=True, perf_mode=DR)

            avs = {}
            wts = {}

            def emit_wv(i):
                rb, j0, njb = seq[i]
                wt = wts.pop(i)
                if j0 == 0:
                    avs[rb] = avp.tile([128, 2, D], F32, name=f"av{rb}", tag="av")
                av = avs[rb]
                for pair in range(njb // 2):
                    jp = j0 // 2 + pair
                    st = (j0 == 0 and pair == 0)
                    sp = (j0 + njb == 4 * NG and pair == njb // 2 - 1)
                    for s in range(2):
                        wsl = wt[:, pair * 2:pair * 2 + 2, s * 128:(s + 1) * 128]
                        nc.tensor.matmul(av[:, s, :], wsl, v8[:, jp, :, :],
                                         start=st, stop=sp, perf_mode=DR)
                        nc.tensor.matmul(den[:, rb * 2 + s:rb * 2 + s + 1],
                                         wsl, ones8[:],
                                         start=st, stop=sp, perf_mode=DR)
                if j0 + njb == 4 * NG:
                    # split the evacuation across both engines (separate
                    # tiles - a shared tile's write-write dep would
                    # serialize them) to halve the latency before the next
                    # row block may reuse the av bank
                    o0 = avsp.tile([128, 1, D + 1], F32, name=f"avsa{rb}", tag="avsa")
                    o1 = avsp.tile([128, 1, D + 1], F32, name=f"avsb{rb}", tag="avsb")
                    nc.vector.tensor_copy(o0[:, 0, 0:D], av[:, 0, :])
                    nc.vector.tensor_copy(o0[:, 0, D:D + 1], den[:, rb * 2:rb * 2 + 1])
                    nc.scalar.activation(o1[:, 0, 0:D], av[:, 1, :], AF.Copy)
                    nc.scalar.activation(o1[:, 0, D:D + 1],
                                         den[:, rb * 2 + 1:rb * 2 + 2], AF.Copy)
                    nc.sync.dma_start(av_d[rb, :, 0:1, :], o0[:])
                    nc.gpsimd.dma_start(av_d[rb, :, 1:2, :], o1[:])

            # Lag the w@v emission behind the relu stream so PE's in-order
            # queue isn't head-blocked by a stalled w@v; the last group of a
            # row block uses lag 1 so the evacuation (which the next row
            # block's accumulator WAR-waits on) isn't itself delayed.
            def wv_lag(i):
                return 1 if seq[i][1] + seq[i][2] == 4 * NG else WVLAG_BASE

            wv_next = 0
            emit_smm(0)
            emit_smm(1)
            for i, (rb, j0, njb) in enumerate(seq):
                sc = sc_tiles.pop(i)
                wt = wtp.tile([128, njb, 256], FP8, name="wt", tag="wt")
                wts[i] = wt
                if i in dve_relu:
                    nc.vector.tensor_scalar_max(wt[:], sc[:], 0.0)
                else:
                    nc.scalar.activation(wt[:], sc[:], AF.Relu)
                if i + 2 < len(seq):
                    emit_smm(i + 2)
                while wv_next <= i and i - wv_next >= wv_lag(wv_next):
                    emit_wv(wv_next)
                    wv_next += 1
            while wv_next < len(seq):
                emit_wv(wv_next)
                wv_next += 1
    nc.compile()
    return nc


def _get_nc(r=N // M):
    if r not in _CACHE:
        _CACHE[r] = build(r)
    return _CACHE[r]


def _to_dr(a2d):
    c, f = a2d.shape
    return np.ascontiguousarray(a2d.reshape(2, 128, f).transpose(1, 0, 2))


def kernel(x, Wq, bq, Wk, bk, Wv, bv):
    global LAST
    np8 = mybir.dt.np(FP8)
    x = np.asarray(x, np.float32)
    n = x.shape[0]
    r = n // M

    q = x @ np.asarray(Wq, np.float32).T + np.asarray(bq, np.float32)
    k = x @ np.asarray(Wk, np.float32).T + np.asarray(bk, np.float32)
    v = x @ np.asarray(Wv, np.float32).T                      # bias added at the end
    kn = k / np.maximum(np.linalg.norm(k, axis=1, keepdims=True), 1e-12)

    q8 = q.astype(np8)
    k8 = kn.astype(np8)
    v8q = v.astype(np8)

    kT8 = _to_dr(np.ascontiguousarray(k8.T))
    v8t = np.ascontiguousarray(v8q.reshape(n // 256, 2, 128, D).transpose(2, 0, 1, 3))

    in_maps = []
    for c in range(M):
        rows = slice(c * r, (c + 1) * r)
        in_maps.append({
            "qT8": _to_dr(np.ascontiguousarray(q8[rows].T)),
            "kT8": kT8,
            "v8": v8t,
        })
    res = run_bass_kernel_spmd(_get_nc(r), in_maps, core_ids=list(range(M)), trace=TRACE)
    LAST = res

    av = np.concatenate(
        [res.results[c]["av"].transpose(0, 2, 1, 3).reshape(r, D + 1) for c in range(M)],
        axis=0).astype(np.float32)
    num = av[:, 0:D]
    den = av[:, D]

    # Remove the diagonal term, emulating the device's fp8 rounding of the
    # relu'd score and of v so the subtraction cancels what was accumulated.
    sdiag = np.maximum((q8.astype(np.float32) * k8.astype(np.float32)).sum(axis=1), 0.0)
    wdiag = sdiag.astype(np8).astype(np.float32)
    num = num - wdiag[:, None] * v8q.astype(np.float32)
    den = den - wdiag

    out = num / np.maximum(den, 1e-12)[:, None] + x + np.asarray(bv, np.float32)
    return np.ascontiguousarray(out, dtype=np.float32)


# revision 52
# speedup vs baseline: 1.0382x; 1.0069x over previous
"""Row-sharded attention slab kernel, host-prepped fp8 Q/K/V.

Each of the 8 cores owns a [N/8, N] slab of the attention matrix.  The
host precomputes the three D x D projections in fp32 (O(N*D^2), ~5% of
total FLOPs), pre-normalizes k rows, and ships fp8 tensors: qT8 (own
rows), kT8 (all columns, pre-normalized), and v8 (row-major V).

The device does the O(N^2*D) work: score matmuls (fp8 DoubleRow),
ReLU + fp8 cast of the [N/8, N] score slab (split across ACT and DVE,
the true bottleneck), and the w@v accumulation into PSUM.  The row-sum
denominator accumulates in a separate 1-bank PSUM tile via tiny
wt @ ones matmuls, which frees enough PSUM banks to triple-buffer the
score tiles - without that, the WAR edge (score matmuls for group g+2
overwriting the tile relu(g) reads) serializes the pipeline.  Emission
is software-pipelined so those prefetch matmuls precede the w@v
matmuls of the current group in PE order.

The host then removes the diagonal term (emulating the device's fp8
rounding so the subtraction matches what the device accumulated),
divides by the row sums, and adds the residual x and the V bias (the
bias commutes out of the attention average because rows of w sum to 1).
"""

import numpy as np

import concourse.bass as bass
import concourse.bacc as bacc
import concourse.mybir as mybir
from concourse import tile
from concourse.bass_utils import run_bass_kernel_spmd

F32 = mybir.dt.float32
FP8 = mybir.dt.float8e4
AF = mybir.ActivationFunctionType
DR = mybir.MatmulPerfMode.DoubleRow

M = 8
N = 8192
D = 256

TRACE = False
LAST = None
_CACHE = {}

# ~45% of the relu tiles go to DVE (1192ns/op) and the rest to ACT
# (1067ns/op) so both engines finish together.
N_DVE_RELU = 29
# number of half-width (2 col-block) groups at the start of rb0 / end of
# the last rb: shortens pipeline fill and drain at the cost of one extra
# per-op bubble each
N_SMALL_HEAD = 0
N_SMALL_TAIL = 0
WVLAG_BASE = 2
WT_BUFS = 8


DVE_PHASE = 0.46
# engine assignment found by randomized local search over the cost model
DVE_SET = {1, 3, 5, 7, 9, 12, 14, 16, 18, 20, 23, 25, 27, 29, 31, 33,
           36, 38, 40, 42, 44, 47, 49, 51, 53, 56, 58, 60, 62}


def _dve_set(total, n_dve):
    s = {int((j + DVE_PHASE) * total / n_dve) % total for j in range(n_dve)}
    while len(s) < n_dve:
        s.add(max(set(range(total)) - s))
    # keep the final relus alternating so both engines finish together
    if total - 2 not in s:
        s.discard(max(i for i in s if i < total - 2))
        s.add(total - 2)
    return s


def build(r=N // M):
    NP_ = N // 256           # 32 column pairs (v8 major dim)
    NCH = N // 1024          # 8 streaming chunks for kT8/v8
    NG = N // 512            # 16 score groups (4 col-blocks each) per row block
    RW = 256
    NRB = r // RW            # 4 row blocks

    nc = bacc.Bacc(None)
    # kq layout: [ k cols 0:512 | q rows 0:r | k cols 512:N ]
    kq_d = nc.declare_dram_parameter("kq", [128, 2, N + r], FP8, isOutput=False)
    v8_d = nc.declare_dram_parameter("v8", [128, NP_, 2, D], FP8, isOutput=False)
    # av_d[rb, p, s, :] = [num | den] for row rb*256 + s*128 + p
    av_d = nc.declare_dram_parameter("av", [NRB, 128, 2, D + 1], F32, isOutput=True)

    # groups: (rb, first col-block, n col-blocks); narrow groups at the two
    # ends of the run shorten fill/drain
    seq = []
    for rb in range(NRB):
        jbs = []
        j = 0
        if rb == 0:
            for _ in range(N_SMALL_HEAD):
                jbs.append((j, 2))
                j += 2
        tail = [];
        jt = 4 * NG
        if rb == NRB - 1:
            for _ in range(N_SMALL_TAIL):
                jt -= 2
                tail.append((jt, 2))
            tail.reverse()
        while j < jt:
            jbs.append((j, 4))
            j += 4
        for j0, njb in jbs + tail:
            seq.append((rb, j0, njb))
    dve_relu = DVE_SET if DVE_SET is not None else _dve_set(len(seq), N_DVE_RELU)

    with tile.TileContext(nc, pool_alloc_mode="queue") as tc:
        with tc.tile_pool(name="pers", bufs=1) as pers, \
             tc.tile_pool(name="wtp", bufs=WT_BUFS) as wtp, \
             tc.tile_pool(name="avsp", bufs=2) as avsp, \
             tc.tile_pool(name="scp", bufs=3, space="PSUM") as scp, \
             tc.tile_pool(name="avp", bufs=1, space="PSUM") as avp, \
             tc.tile_pool(name="denp", bufs=1, space="PSUM") as denp:
            kq = pers.tile([128, 2, N + r], FP8, name="kq", tag="kq")
            v8 = pers.tile([128, NP_, 2, D], FP8, name="v8", tag="v8")
            ones8 = pers.tile([128, 2, 1], FP8, name="ones8", tag="ones8")
            den = denp.tile([128, 2 * NRB], F32, name="den", tag="den")

            nc.vector.memset(ones8[:], 1.0)
            # warm up the ACT Relu table during the DMA fill so the ~1.3us
            # table load isn't charged to the first real score evacuation
            warm = pers.tile([128, 1, 1], FP8, name="warm", tag="warm")
            nc.scalar.activation(warm[:], ones8[:, 0:1, 0:1], AF.Relu)

            # Stream inputs on two issue queues (SP: kT8/qT8, Pool: v8) so
            # chunk delivery outpaces the ~570ns/group compute consumption;
            # small first chunks let group 0 start early.
            def kqdma(a, b):
                nc.sync.dma_start(kq[:, :, a:b], kq_d[:, :, a:b])

            kqdma(0, 512 + RW)                       # k[0:512] + q rows of rb0
            nc.gpsimd.dma_start(v8[:, 0:2, :, :], v8_d[:, 0:2, :, :])
            nc.gpsimd.dma_start(v8[:, 2:4, :, :], v8_d[:, 2:4, :, :])
            kqdma(512 + r, 1024 + r)                 # k[512:1024]
            kqdma(1024 + r, 2048 + r)                # k[1024:2048]
            kqdma(2048 + r, 3072 + r)                # k[2048:3072]
            kqdma(512 + RW, 512 + r)                 # q rows of rb1..3
            for ch in range(3, NCH):
                kqdma(ch * 1024 + r, (ch + 1) * 1024 + r)
            for ch in range(1, NCH):
                nc.gpsimd.dma_start(v8[:, ch * 4:(ch + 1) * 4, :, :],
                                    v8_d[:, ch * 4:(ch + 1) * 4, :, :])

            sc_tiles = {}

            def emit_smm(i):
                rb, j0, njb = seq[i]
                sc = scp.tile([128, njb * 256], F32, name="sc", tag="sc")
                sc_tiles[i] = sc
                rsl = slice(rb * RW, (rb + 1) * RW)
                qsl = slice(512 + rsl.start, 512 + rsl.stop)
                for t in range(njb):
                    ko = (j0 + t) * 128 + (0 if j0 + t < 4 else r)
                    nc.tensor.matmul(sc[:, t * 256:(t + 1) * 256],
                                     kq[:, :, ko:ko + 128],
                                     kq[:, :, qsl],
                                     start=True, stop=True, perf_mode=DR)

            avs = {}
            wts = {}

            def emit_wv(i):
                rb, j0, njb = seq[i]
                wt = wts.pop(i)
                if j0 == 0:
                    avs[rb] = avp.tile([128, 2, D], F32, name=f"av{rb}", tag="av")
                av = avs[rb]
                for pair in range(njb // 2):
                    jp = j0 // 2 + pair
                    st = (j0 == 0 and pair == 0)
                    sp = (j0 + njb == 4 * NG and pair == njb // 2 - 1)
                    for s in range(2):
                        wsl = wt[:, pair * 2:pair * 2 + 2, s * 128:(s + 1) * 128]
                        nc.tensor.matmul(av[:, s, :], wsl, v8[:, jp, :, :],
                                         start=st, stop=sp, perf_mode=DR)
                        nc.tensor.matmul(den[:, rb * 2 + s:rb * 2 + s + 1],
                                         wsl, ones8[:],
                                         start=st, stop=sp, perf_mode=DR)
                if j0 + njb == 4 * NG:
                    # split the evacuation across both engines (separate
                    # tiles - a shared tile's write-write dep would
                    # serialize them) to halve the latency before the next
                    # row block may reuse the av bank
                    o0 = avsp.tile([128, 1, D + 1], F32, name=f"avsa{rb}", tag="avsa")
                    o1 = avsp.tile([128, 1, D + 1], F32, name=f"avsb{rb}", tag="avsb")
                    nc.vector.tensor_copy(o0[:, 0, 0:D], av[:, 0, :])
                    nc.vector.tensor_copy(o0[:, 0, D:D + 1], den[:, rb * 2:rb * 2 + 1])
                    nc.scalar.activation(o1[:, 0, 0:D], av[:, 1, :], AF.Copy)
                    nc.scalar.activation(o1[:, 0, D:D + 1],
                                         den[:, rb * 2 + 1:rb * 2 + 2], AF.Copy)
                    nc.sync.dma_start(av_d[rb, :, 0:1, :], o0[:])
                    nc.gpsimd.dma_start(av_d[rb, :, 1:2, :], o1[:])

            # Lag the w@v emission behind the relu stream so PE's in-order
            # queue isn't head-blocked by a stalled w@v; the last group of a
            # row block uses lag 1 so the evacuation (which the next row
            # block's accumulator WAR-waits on) isn't itself delayed.
            def wv_lag(i):
                return 1 if seq[i][1] + seq[i][2] == 4 * NG else WVLAG_BASE

            wv_next = 0
            emit_smm(0)
            emit_smm(1)
            for i, (rb, j0, njb) in enumerate(seq):
                sc = sc_tiles.pop(i)
                wt = wtp.tile([128, njb, 256], FP8, name="wt", tag="wt")
                wts[i] = wt
                if i in dve_relu:
                    nc.vector.tensor_scalar_max(wt[:], sc[:], 0.0)
                else:
                    nc.scalar.activation(wt[:], sc[:], AF.Relu)
                if i + 2 < len(seq):
                    emit_smm(i + 2)
                while wv_next <= i and i - wv_next >= wv_lag(wv_next):
                    emit_wv(wv_next)
                    wv_next += 1
            while wv_next < len(seq):
                emit_wv(wv_next)
                wv_next += 1
    nc.compile()
    return nc


def _get_nc(r=N // M):
    if r not in _CACHE:
        _CACHE[r] = build(r)
    return _CACHE[r]


def _to_dr(a2d):
    c, f = a2d.shape
    return np.ascontiguousarray(a2d.reshape(2, 128, f).transpose(1, 0, 2))


def kernel(x, Wq, bq, Wk, bk, Wv, bv):
    global LAST
    np8 = mybir.dt.np(FP8)
    x = np.asarray(x, np.float32)
    n = x.shape[0]
    r = n // M

    q = x @ np.asarray(Wq, np.float32).T + np.asarray(bq, np.float32)
    k = x @ np.asarray(Wk, np.float32).T + np.asarray(bk, np.float32)
    v = x @ np.asarray(Wv, np.float32).T                      # bias added at the end
    kn = k / np.maximum(np.linalg.norm(k, axis=1, keepdims=True), 1e-12)

    q8 = q.astype(np8)
    k8 = kn.astype(np8)
    v8q = v.astype(np8)

    kT8 = _to_dr(np.ascontiguousarray(k8.T))
    v8t = np.ascontiguousarray(v8q.reshape(n // 256, 2, 128, D).transpose(2, 0, 1, 3))

    in_maps = []
    for c in range(M):
        rows = slice(c * r, (c + 1) * r)
        qT8 = _to_dr(np.ascontiguousarray(q8[rows].T))
        kq = np.ascontiguousarray(
            np.concatenate([kT8[:, :, 0:512], qT8, kT8[:, :, 512:]], axis=2))
        in_maps.append({
            "kq": kq,
            "v8": v8t,
        })
    res = run_bass_kernel_spmd(_get_nc(r), in_maps, core_ids=list(range(M)), trace=TRACE)
    LAST = res

    av = np.concatenate(
        [res.results[c]["av"].transpose(0, 2, 1, 3).reshape(r, D + 1) for c in range(M)],
        axis=0).astype(np.float32)
    num = av[:, 0:D]
    den = av[:, D]

    # Remove the diagonal term, emulating the device's fp8 rounding of the
    # relu'd score and of v so the subtraction cancels what was accumulated.
    sdiag = np.maximum((q8.astype(np.float32) * k8.astype(np.float32)).sum(axis=1), 0.0)
    wdiag = sdiag.astype(np8).astype(np.float32)
    num = num - wdiag[:, None] * v8q.astype(np.float32)
    den = den - wdiag

    out = num / np.maximum(den, 1e-12)[:, None] + x + np.asarray(bv, np.float32)
    return np.ascontiguousarray(out, dtype=np.float32)
